# revision 1
# baseline (speedup 1.0000x reference)
"""DRNL filterbank Trainium2 kernel.

Math: the reference pipeline is
    xs    = x * 10**((93.98-100)/20)
    xme   = causal_fir(xs, me_fir)                                  (shared, 512 taps)
    y_lin = lfilter^4(causal_fir(lin_gain*xme, lin_fir), lpf_lin)   (per channel)
    v     = causal_fir(xme, nlin_fir_before)                        (per channel)
    w     = sign(v)*min(a|v|, b*max(|v|,eps)^0.25)
    y_nl  = lfilter^3(causal_fir(w, nlin_fir_after), lpf_nlin)      (per channel)
    out   = y_lin + y_nl

All IIR lowpass cascades are LTI per channel, so they are folded on the host into
truncated FIR impulse responses combined with the gammatone FIRs:
    LIN_f = lin_gain_f * (lin_fir_f (*) irlpf4_f)     (truncated)
    BEF_f = nlin_fir_f                                 (truncated)
    AFT_f = nlin_fir_f (*) irlpf3_f                    (truncated)
On device everything is 3 FIR stages + the pointwise broken-stick, evaluated as
banded-Toeplitz matmuls in fold-128 layout: for a signal u folded as
u_fold[k, r] = u[128 r + k], the causal FIR y = h (*) u is
    y_fold[i, r] = sum_d  Td[k, i]^T  u_fold[k, r - d],   Td[k, i] = h[128 d + i - k]
which maps directly onto the 128x128 tensor engine (contraction over k).

Sharding: channels are split across the 8 cores (slot-structured SPMD: every core
runs the identical program; per-core DRAM data carries its channels' Toeplitz
weights). Each core computes all 8 batch items for its ~6-7 channels.
Matmuls run in float32r (full-rate, ~1.5e-4 matmul rel err on HW).
"""
import numpy as np

P = 128
B, T, F = 8, 20000, 50
R = (T + P - 1) // P          # 157 fold blocks
EPS = 1e-12
N_CORES = 8
N_SLOTS = 7                    # 6 full slots x 8 cores + slot 6 (2 real + 6 dummy)
TRUNC_LIN = 3e-5   # linear path dominates output amplitude (~68 of 86 absmax)
TRUNC_NL = 1e-3    # nonlinear path is ~500x smaller (absmax ~0.15)
IR_LEN = 4096

_CACHE = {}


# ----------------------------------------------------------------- host math
def _lfilter_vec(x, b, a):
    """DFII-transposed biquad, float64; x: (F, T), b/a: (F, 3)."""
    b0, b1, b2 = b[:, 0], b[:, 1], b[:, 2]
    a1, a2 = a[:, 1], a[:, 2]
    y = np.zeros_like(x)
    z1 = np.zeros(x.shape[0])
    z2 = np.zeros(x.shape[0])
    for t in range(x.shape[-1]):
        xt = x[:, t]
        yt = b0 * xt + z1
        z1 = b1 * xt - a1 * yt + z2
        z2 = b2 * xt - a2 * yt
        y[:, t] = yt
    return y


def _cascade_ir(b, a, n, times):
    h = np.zeros((b.shape[0], n))
    h[:, 0] = 1.0
    for _ in range(times):
        h = _lfilter_vec(h, b, a)
    return h


def _trunc(h, tol):
    m = np.abs(h).max()
    idx = np.nonzero(np.abs(h) > tol * m)[0]
    return h[: int(idx[-1]) + 1] if len(idx) else h[:1]


def _toeplitz_bands(h, nb):
    """h: (K,) -> W: (128, nb*128) with W[k, 128 d + i] = h[128 d + i - k]."""
    K = len(h)
    d = np.arange(nb)[:, None, None]
    k = np.arange(P)[None, :, None]
    i = np.arange(P)[None, None, :]
    idx = P * d + i - k
    ok = (idx >= 0) & (idx < K)
    W = np.where(ok, np.asarray(h, np.float64)[np.clip(idx, 0, K - 1)], 0.0)
    return np.ascontiguousarray(W.transpose(1, 0, 2).reshape(P, nb * P)).astype(np.float32)


def _build_host(me_fir, lin_fir, nlin_fir_before, nlin_fir_after,
                lpf_lin_b, lpf_lin_a, lpf_nlin_b, lpf_nlin_a,
                lin_gain, nlin_a, nlin_b):
    """All data-independent preprocessing: combined filters, Toeplitz weights,
    slot assignment. Returns a dict of per-core DRAM arrays + layout metadata."""
    ir4 = _cascade_ir(lpf_lin_b.astype(np.float64), lpf_lin_a.astype(np.float64), IR_LEN, 4)
    ir3 = _cascade_ir(lpf_nlin_b.astype(np.float64), lpf_nlin_a.astype(np.float64), IR_LEN, 3)

    scale = 10.0 ** ((93.98 - 100.0) / 20.0)
    ME = np.asarray(me_fir, np.float64) * scale

    LIN, BEF, AFT = [], [], []
    for f in range(F):
        LIN.append(_trunc(lin_gain[f] * np.convolve(np.asarray(lin_fir[f], np.float64), ir4[f]), TRUNC_LIN))
        BEF.append(_trunc(np.asarray(nlin_fir_before[f], np.float64), TRUNC_NL))
        AFT.append(_trunc(np.convolve(np.asarray(nlin_fir_after[f], np.float64), ir3[f]), TRUNC_NL))

    # bands for a K-tap filter: need all d with 128 d - 127 <= K - 1
    nb = lambda h: (len(h) + P - 2) // P + 1
    cost = [nb(LIN[f]) + nb(BEF[f]) + nb(AFT[f]) for f in range(F)]
    order = np.argsort(-np.asarray(cost), kind="stable")  # heavy first

    # slots 0..5: channel groups of 8 (one per core); slot 6: two cheapest channels
    # on cores 0-1, duplicated (ignored) on cores 2-7.
    slot_ch = np.zeros((N_CORES, N_SLOTS), np.int64)
    for s in range(6):
        for c in range(N_CORES):
            slot_ch[c, s] = order[8 * s + c]
    for c in range(N_CORES):
        slot_ch[c, 6] = order[48 + (c % 2)]

    BL = [max(nb(LIN[slot_ch[c, s]]) for c in range(N_CORES)) for s in range(N_SLOTS)]
    BB = [max(nb(BEF[slot_ch[c, s]]) for c in range(N_CORES)) for s in range(N_SLOTS)]
    BA = [max(nb(AFT[slot_ch[c, s]]) for c in range(N_CORES)) for s in range(N_SLOTS)]

    wme = _toeplitz_bands(ME, (len(ME) + P - 2) // P + 1)
    wlin = [np.concatenate([_toeplitz_bands(LIN[slot_ch[c, s]], BL[s]) for s in range(N_SLOTS)], axis=1)
            for c in range(N_CORES)]
    wbef = [np.concatenate([_toeplitz_bands(BEF[slot_ch[c, s]], BB[s]) for s in range(N_SLOTS)], axis=1)
            for c in range(N_CORES)]
    waft = [np.concatenate([_toeplitz_bands(AFT[slot_ch[c, s]], BA[s]) for s in range(N_SLOTS)], axis=1)
            for c in range(N_CORES)]

    scal = np.zeros((N_CORES, N_SLOTS * 4), np.float32)
    for c in range(N_CORES):
        for s in range(N_SLOTS):
            f = slot_ch[c, s]
            scal[c, 4 * s + 0] = nlin_a[f]
            scal[c, 4 * s + 1] = float(nlin_b[f]) ** 4

    return {
        "slot_ch": slot_ch, "BL": BL, "BB": BB, "BA": BA,
        "BME": wme.shape[1] // P,
        "wme": wme, "wlin": wlin, "wbef": wbef, "waft": waft, "scal": scal,
    }


def _pads(meta):
    BL, BB, BA, BME = meta["BL"], meta["BB"], meta["BA"], meta["BME"]
    PADX = (BME - 1 + 1) // 2 * 2
    PADS = (max(max(BL), max(BB), max(BA)) - 1 + 1) // 2 * 2
    return PADX, PADS


def _fold_x(x, padx):
    """(B, T) fp32 -> (B, 128, padx + R) with padx zero cols in front."""
    xp = np.zeros((B, R * P), np.float32)
    xp[:, :T] = x
    xf = np.zeros((B, P, padx + R), np.float32)
    xf[:, :, padx:] = xp.reshape(B, R, P).transpose(0, 2, 1)
    return xf


# ------------------------------------------------------------- device program
def _build_program(meta):
    import concourse.bacc as bacc
    from concourse import mybir
    from concourse.tile import TileContext

    BL, BB, BA, BME = meta["BL"], meta["BB"], meta["BA"], meta["BME"]
    SBL, SBB, SBA = sum(BL), sum(BB), sum(BA)
    # pads rounded up to even so 2-batch matmul windows have even width
    # (fp32r ISA: moving-operand innermost count must be even)
    PADX, PADS = _pads(meta)
    XSEC = PADX + R                 # xf per-batch section width
    SEC = PADS + R                  # xme / w per-batch section width
    XW = XSEC + R                   # 2-batch window width (ME stage)
    WW = SEC + R                    # 2-batch window width (main stages)
    f32, f32r = mybir.dt.float32, mybir.dt.float32r

    nc = bacc.Bacc("TRN2", target_bir_lowering=False, debug=False, num_devices=N_CORES)
    d_xf = nc.dram_tensor("xf", [B, P, XSEC], f32r, kind="ExternalInput").ap()
    d_wme = nc.dram_tensor("wme", [P, BME * P], f32r, kind="ExternalInput").ap()
    d_wlin = nc.dram_tensor("wlin", [P, SBL * P], f32r, kind="ExternalInput").ap()
    d_wbef = nc.dram_tensor("wbef", [P, SBB * P], f32r, kind="ExternalInput").ap()
    d_waft = nc.dram_tensor("waft", [P, SBA * P], f32r, kind="ExternalInput").ap()
    d_scal = nc.dram_tensor("scal", [N_SLOTS * 4], f32, kind="ExternalInput").ap()
    d_out = nc.dram_tensor("yout", [N_SLOTS, 2, P, 2 * WW], f32, kind="ExternalOutput").ap()

    import concourse.bass as bass

    with TileContext(nc) as tc:
        with (
            tc.tile_pool(name="singles", bufs=1) as singles,
            tc.tile_pool(name="work", bufs=3) as work,
            tc.tile_pool(name="ps", bufs=2, space="PSUM") as ps,
        ):
            # resident inputs
            xf_t = singles.tile([P, B * XSEC], f32r)
            nc.sync.dma_start(out=xf_t.rearrange("k (b c) -> k b c", b=B),
                              in_=d_xf.rearrange("b k c -> k b c"))
            wme_t = singles.tile([P, BME * P], f32r)
            nc.sync.dma_start(out=wme_t, in_=d_wme)
            scal_t = singles.tile([P, N_SLOTS * 4], f32)
            nc.sync.dma_start(
                out=scal_t,
                in_=bass.AP(tensor=d_scal.tensor, offset=d_scal.offset,
                            ap=[[0, P], [1, N_SLOTS * 4]]),
            )
            wl_t, wb_t, wa_t = [], [], []
            ol = ob = oa = 0
            for s in range(N_SLOTS):
                t = singles.tile([P, BL[s] * P], f32r, tag=f"wl{s}")
                nc.sync.dma_start(out=t, in_=d_wlin[:, ol * P:(ol + BL[s]) * P])
                wl_t.append(t); ol += BL[s]
                t = singles.tile([P, BB[s] * P], f32r, tag=f"wb{s}")
                nc.sync.dma_start(out=t, in_=d_wbef[:, ob * P:(ob + BB[s]) * P])
                wb_t.append(t); ob += BB[s]
                t = singles.tile([P, BA[s] * P], f32r, tag=f"wa{s}")
                nc.sync.dma_start(out=t, in_=d_waft[:, oa * P:(oa + BA[s]) * P])
                wa_t.append(t); oa += BA[s]

            # middle-ear stage -> xme (all 8 batches), per-section left zero pad
            xme_t = singles.tile([P, B * SEC], f32r)
            nc.vector.memset(xme_t.bitcast(f32), 0.0)
            for ci in range(4):
                q = 2 * ci
                mp = ps.tile([P, XW], f32, tag=f"o{ci % 2}")
                for d in range(BME):
                    ws = q * XSEC + PADX - d
                    nc.tensor.matmul(mp, wme_t[:, d * P:(d + 1) * P],
                                     xf_t[:, ws:ws + XW],
                                     start=(d == 0), stop=(d == BME - 1))
                nc.vector.tensor_copy(out=xme_t[:, q * SEC + PADS:(q + 1) * SEC],
                                      in_=mp[:, 0:R])
                nc.vector.tensor_copy(out=xme_t[:, (q + 1) * SEC + PADS:(q + 2) * SEC],
                                      in_=mp[:, XSEC:XW])

            # main per-(slot, half) loop; heavy slots first — their long matmul
            # streams hide the pointwise latency of neighbouring iterations
            for s in range(N_SLOTS):
                a_ap = scal_t[:, 4 * s + 0:4 * s + 1]
                b4_ap = scal_t[:, 4 * s + 1:4 * s + 2]
                for h in range(2):
                    w_t = work.tile([P, 4 * SEC], f32r, tag="w")
                    nc.vector.memset(w_t[:, 0:PADS].bitcast(f32), 0.0)
                    nc.vector.memset(w_t[:, 2 * SEC:2 * SEC + PADS].bitcast(f32), 0.0)
                    o_ps = []
                    for ci in range(2):
                        q = 4 * h + 2 * ci        # xme section of first batch in chunk
                        # nonlinear-path input conv: v = BEF (*) xme
                        v_ps = ps.tile([P, WW], f32, tag=f"v{ci}")
                        for d in range(BB[s]):
                            ws = q * SEC + PADS - d
                            nc.tensor.matmul(v_ps, wb_t[s][:, d * P:(d + 1) * P],
                                             xme_t[:, ws:ws + WW],
                                             start=(d == 0), stop=(d == BB[s] - 1))
                        # broken-stick pointwise: w = clip(a*v, +-b*|v|^0.25)
                        # (identical to the reference's max(|v|,eps) form: wherever
                        # |v| < eps the a|v| arm of the min wins in both variants)
                        t2 = work.tile([P, WW], f32, tag="t2")
                        nc.scalar.activation(t2, v_ps, mybir.ActivationFunctionType.Abs)
                        c_t = work.tile([P, WW], f32, tag="c")
                        nc.scalar.activation(c_t, t2, mybir.ActivationFunctionType.Sqrt,
                                             scale=b4_ap)
                        nc.scalar.sqrt(c_t, c_t)
                        m_t = work.tile([P, WW], f32, tag="m")
                        nc.vector.scalar_tensor_tensor(
                            out=m_t, in0=v_ps, scalar=a_ap, in1=c_t,
                            op0=mybir.AluOpType.mult, op1=mybir.AluOpType.min,
                        )
                        nc.vector.scalar_tensor_tensor(
                            out=w_t[:, 2 * ci * SEC + PADS:(2 * ci + 2) * SEC],
                            in0=c_t, scalar=-1.0, in1=m_t,
                            op0=mybir.AluOpType.mult, op1=mybir.AluOpType.max,
                        )
                        # re-zero the mid pad the pointwise junk just overwrote
                        nc.vector.memset(
                            w_t[:, (2 * ci + 1) * SEC:(2 * ci + 1) * SEC + PADS].bitcast(f32), 0.0)
                        # linear path conv accumulates into the output psum
                        op = ps.tile([P, WW], f32, tag=f"o{ci}")
                        o_ps.append(op)
                        for d in range(BL[s]):
                            ws = q * SEC + PADS - d
                            nc.tensor.matmul(op, wl_t[s][:, d * P:(d + 1) * P],
                                             xme_t[:, ws:ws + WW],
                                             start=(d == 0), stop=False)
                    # nonlinear-path output conv accumulates on top of y_lin
                    out_t = work.tile([P, 2 * WW], f32, tag="out")
                    for ci in range(2):
                        for d in range(BA[s]):
                            ws = 2 * ci * SEC + PADS - d
                            nc.tensor.matmul(o_ps[ci], wa_t[s][:, d * P:(d + 1) * P],
                                             w_t[:, ws:ws + WW],
                                             start=False, stop=(d == BA[s] - 1))
                        # single copy including the junk mid-columns; host skips them
                        nc.vector.tensor_copy(out=out_t[:, ci * WW:(ci + 1) * WW],
                                              in_=o_ps[ci])
                    nc.sync.dma_start(out=d_out[s, h], in_=out_t)
    nc.compile()
    return nc


def kernel(**inputs):
    x = np.asarray(inputs["x"], np.float32)
    key = "prog"
    if key not in _CACHE:
        meta = _build_host(
            inputs["me_fir"], inputs["lin_fir"], inputs["nlin_fir_before"],
            inputs["nlin_fir_after"], inputs["lpf_lin_b"], inputs["lpf_lin_a"],
            inputs["lpf_nlin_b"], inputs["lpf_nlin_a"],
            np.asarray(inputs["lin_gain"], np.float64),
            np.asarray(inputs["nlin_a"], np.float64),
            np.asarray(inputs["nlin_b"], np.float64),
        )
        _CACHE[key] = (meta, _build_program(meta))
    meta, nc = _CACHE[key]

    from concourse.bass_utils import run_bass_kernel_spmd

    xf = _fold_x(x, _pads(meta)[0])
    in_maps = [
        {"xf": xf, "wme": meta["wme"], "wlin": meta["wlin"][c], "wbef": meta["wbef"][c],
         "waft": meta["waft"][c], "scal": meta["scal"][c]}
        for c in range(N_CORES)
    ]
    res = run_bass_kernel_spmd(nc, in_maps, core_ids=list(range(N_CORES)),
                               trace=bool(inputs.get("_trace", False)))

    out = np.zeros((B, F, T), np.float32)
    slot_ch = meta["slot_ch"]
    _, PADS = _pads(meta)
    SEC = PADS + R
    WW = SEC + R
    for c in range(N_CORES):
        # yout: (slot, half, k, 2*WW); chunk ci cols [ci*WW, (ci+1)*WW):
        # batch 4h+2ci at [0:R], batch 4h+2ci+1 at [SEC:SEC+R], junk between
        yo = res.results[c]["yout"].reshape(N_SLOTS, 2, P, 2, WW)
        for s in range(N_SLOTS):
            if s == 6 and c >= 2:
                continue
            f = slot_ch[c, s]
            y = np.stack([yo[s, :, :, :, 0:R], yo[s, :, :, :, SEC:SEC + R]], axis=3)
            # y: (half, k, ci, bi, r) -> (half, ci, bi, r, k) -> (b, t)
            y = y.transpose(0, 2, 3, 4, 1).reshape(8, R * P)
            out[:, f, :] = y[:, :T]
    if inputs.get("_return_res", False):
        return out, res
    return out

